# revision 31
# baseline (speedup 1.0000x reference)
"""Distributed causal multi-head attention (RoPE) for 8 TRN2 NeuronCores.

Problem: B=4, S=2048, D=2048, H=16 heads, DH=128.
Sharding: 2D - data-parallel over the 4 batches x tensor-parallel over 2
head-groups of 8 heads (Megatron-style: Wqkv column-sharded per head
group, Wo row-sharded).  Core c handles batch c//2, head group c%2.
Each core returns a partial output projection [S, D]; the host sums the
two group partials per batch (the "all-reduce") and stacks batches.

v10: Q/K resident in SBUF as bf16 (no spill); all matmul operands bf16
(FWL fast weight loads); x stored token-quarter-major so loads are
4KB-contiguous per partition, issued critical-first with a progressive
per-4-o (w, x) start; stage-2 exp runs on 1024-wide score tiles (two QK
matmuls fill the two PSUM banks of one [128,1024] tile, one ACT
instruction exps both) since attention is ACT-throughput-bound; heads
0-1's score tiles + exps overlap the V-projection (ACT idle there);
attention runs in two 1024-token halves with the out-projection of half
0 interleaved into half 1's attention stream so the PE never waits on
ACT; PV accumulators and projection psums share a 3-deep PSUM ring;
bf16 PE transposes feed resident zT tiles.

Per-core pipeline:
  stage 1: QKV projection from xT (d-major); RoPE fused into the PSUM
           eviction for Q/K (DVE, rotate_half via cross-partition reads,
           sign folded into the sin table) writing resident bf16 Q/K;
           V evicted as bf16 (N=512) and spilled to DRAM (reloaded per
           head as the PV moving operand with a fused ones-column).
  stage 2+3: per head: scoresT[k,q] tiles [128,1024] (causally
           trimmed), exp via ACT (scale fused) into bf16, triangular
           mask on diagonal blocks, PV with the ones-column giving the
           softmax denominator for free, reciprocal-scale + bf16 PE
           transpose into resident zT; out = sum_h zT_h.T x WoT_h.
"""

import sys

if '/opt/trn_rl_repo' not in sys.path:
    sys.path.insert(0, '/opt/trn_rl_repo')

import math

import ml_dtypes
import numpy as np

B, S, D, H, DH = 4, 2048, 2048, 16, 128
BASE = 10000.0
P = 128
NT = S // P          # 16 token tiles of 128
NC512 = S // 512     # 4 token chunks of 512
NDM = D // P         # 16 d_model chunks
HG = 8               # heads per group
NFBG = 4             # Q/K feature-block groups (4 fb of 128 each)
SCALE = 1.0 / math.sqrt(DH)

_CACHE = {}


def _build_program():
    import concourse.bacc as bacc
    import concourse.mybir as mybir
    from concourse.tile import TileContext

    F32 = mybir.dt.float32
    BF16 = mybir.dt.bfloat16
    EXP = mybir.ActivationFunctionType.Exp

    nc = bacc.Bacc('TRN2', target_bir_lowering=False, debug=False, num_devices=8)

    # ---- DRAM I/O ----
    # x stored token-quarter-major so quarter loads are 4KB-contiguous per
    # partition (full DMA packet efficiency during the critical startup)
    xT = nc.dram_tensor('xT', [P, NC512, NDM, 512], BF16, kind='ExternalInput').ap()
    # Q then K feature blocks grouped 4 at a time, f4-major for per-fb DMA
    wqkT = nc.dram_tensor('wqkT', [NFBG, P, 4, NDM, P], BF16, kind='ExternalInput').ap()
    wvT = nc.dram_tensor('wvT', [P, NDM, HG * P], BF16, kind='ExternalInput').ap()
    woT = nc.dram_tensor('woT', [P, HG, D], BF16, kind='ExternalInput').ap()
    cosT = nc.dram_tensor('cosT', [P, S], F32, kind='ExternalInput').ap()
    sinP = nc.dram_tensor('sinP', [P, S], F32, kind='ExternalInput').ap()
    maskT = nc.dram_tensor('maskT', [P, P], BF16, kind='ExternalInput').ap()
    onesb = nc.dram_tensor('onesb', [P, 1], BF16, kind='ExternalInput').ap()
    identT = nc.dram_tensor('identT', [P, P], BF16, kind='ExternalInput').ap()
    out = nc.dram_tensor('out', [NT, P, D], F32, kind='ExternalOutput').ap()

    # ---- DRAM scratch (bf16) ----
    v_scr = nc.dram_tensor('v_scr', [NT, P, HG * P], BF16).ap()

    with TileContext(nc) as tc:
        with tc.tile_pool(name='s2c', bufs=1) as cpool, \
             tc.tile_pool(name='qkres', bufs=1) as qkrpool:
            msk = cpool.tile([P, P], BF16)
            ident = cpool.tile([P, P], BF16)
            ones_sb = cpool.tile([P, 1], BF16)
            # resident Q/K: [dh=128, fb, S] bf16; fb 0-7 = Q heads, 8-15 = K
            qk_res = qkrpool.tile([P, 2 * HG, S], BF16)

            # ---- early score-tile pool (heads 0-1, first half) + score
            # psum: opened first so they outlive stage 1 (LIFO pool order);
            # their exps overlap the V projection where ACT is idle
            _early_cm = tc.tile_pool(name='s2ste', bufs=1)
            _spp_cm = tc.tile_pool(name='s2p', bufs=2, space='PSUM')
            earlypool = _early_cm.__enter__()
            sppool = _spp_cm.__enter__()
            _stref = []

            # ---- persistent stage-1 pools (closed manually before the
            # SBUF-heavy attention pools open) ----
            _s1 = [tc.tile_pool(name='s1x', bufs=1),
                   tc.tile_pool(name='s1w', bufs=2),
                   tc.tile_pool(name='s1ev', bufs=3)]
            xpool, wpool, evpool = [p.__enter__() for p in _s1]
            xsb = xpool.tile([P, NC512, NDM, 512], BF16)

            # ============ stage 1a: Q/K projection + RoPE ============
            with tc.tile_pool(name='s1cs', bufs=1) as cspool, \
                 tc.tile_pool(name='s1e', bufs=2) as epool, \
                 tc.tile_pool(name='s1p', bufs=4, space='PSUM') as ppool:
                cos_sb = cspool.tile([P, S], F32)
                sin_sb = cspool.tile([P, S], F32)

                def load_x_quarter(tcn):
                    ts0 = slice(tcn * 512, tcn * 512 + 512)
                    for o4 in range(0, NDM, 4):
                        nc.sync.dma_start(xsb[:, tcn, o4:o4 + 4, :],
                                          xT[:, tcn, o4:o4 + 4, :])
                    nc.sync.dma_start(cos_sb[:, ts0], cosT[:, ts0])
                    nc.sync.dma_start(sin_sb[:, ts0], sinP[:, ts0])

                for fbg in range(NFBG):
                    wsb = wpool.tile([P, 4, NDM, P], BF16, tag='w', name=f'wqk{fbg}')
                    if fbg == 0:
                        # critical-first, progressive per-4-o (w, x) pairs so
                        # the first chain starts as soon as chunk 0 lands
                        for o4 in range(0, NDM, 4):
                            nc.sync.dma_start(wsb[:, 0, o4:o4 + 4, :],
                                              wqkT[0][:, 0, o4:o4 + 4, :])
                            nc.sync.dma_start(xsb[:, 0, o4:o4 + 4, :],
                                              xT[:, 0, o4:o4 + 4, :])
                        for f4 in range(1, 4):
                            nc.sync.dma_start(wsb[:, f4], wqkT[0][:, f4])
                        nc.sync.dma_start(cos_sb[:, 0:512], cosT[:, 0:512])
                        nc.sync.dma_start(sin_sb[:, 0:512], sinP[:, 0:512])
                        for tcn in range(1, NC512):
                            load_x_quarter(tcn)
                        nc.sync.dma_start(msk[:], maskT[:])
                        nc.sync.dma_start(ident[:], identT[:])
                        nc.sync.dma_start(ones_sb[:], onesb[:])
                    else:
                        for f4 in range(4):
                            nc.sync.dma_start(wsb[:, f4], wqkT[fbg][:, f4])
                    for tcn in range(NC512):
                        ts = slice(tcn * 512, tcn * 512 + 512)
                        for f4 in range(4):
                            fb = fbg * 4 + f4
                            ps = ppool.tile([P, 512], F32, tag='pqk',
                                            name=f'pqk_{fb}_{tcn}')
                            for o in range(NDM):
                                nc.tensor.matmul(ps[:], wsb[:, f4, o, :],
                                                 xsb[:, tcn, o, :],
                                                 start=(o == 0), stop=(o == NDM - 1))
                            # RoPE fused eviction into resident Q/K
                            t1 = epool.tile([P, 512], F32, tag='t1',
                                            name=f't1_{fb}_{tcn}')
                            t2 = epool.tile([P, 512], F32, tag='t2',
                                            name=f't2_{fb}_{tcn}')
                            nc.vector.tensor_mul(t1[:], ps[:], cos_sb[:, ts])
                            # rotate_half via cross-partition reads
                            # (sign folded into sinP)
                            nc.vector.tensor_mul(t2[0:64, :], ps[64:128, :],
                                                 sin_sb[0:64, ts])
                            nc.vector.tensor_mul(t2[64:128, :], ps[0:64, :],
                                                 sin_sb[64:128, ts])
                            nc.vector.tensor_add(qk_res[:, fb, ts], t1[:], t2[:])

            def qk_tile(hd, qr2, kt, stp=None):
                # scoresT tile [k=128, q=1024]: left/right 512-col matmuls
                # into the two PSUM banks, one exp over the causal region
                h = hd['h']
                d2 = kt - 8 * qr2
                base = qr2 * 1024
                eoff = max(0, 128 * d2)
                sps = sppool.tile([P, 1024], F32, tag='sps',
                                  name=f'sps_{h}_{qr2}_{kt}')
                ktb = qk_res[:, HG + h, kt * P:(kt + 1) * P]
                if d2 < 4:
                    nc.tensor.matmul(sps[:, eoff:512], ktb,
                                     qk_res[:, h, base + eoff:base + 512],
                                     start=True, stop=True)
                qoffr = max(512, eoff)
                nc.tensor.matmul(sps[:, qoffr:1024], ktb,
                                 qk_res[:, h, base + qoffr:base + 1024],
                                 start=True, stop=True)
                stt = (stp if stp is not None else _stref[0]).tile(
                    [P, 1024], BF16, tag=f'st{kt}', name=f'st_{h}_{qr2}_{kt}')
                nc.scalar.activation(stt[:, eoff:1024], sps[:, eoff:1024],
                                     EXP, scale=SCALE)
                if d2 >= 0:
                    # triangular mask on the diagonal 128-block
                    nc.vector.tensor_mul(stt[:, eoff:eoff + P],
                                         stt[:, eoff:eoff + P], msk[:])
                hd['st'][kt] = stt

            # ============ stage 1b: V projection + early score tiles ========
            early = [{'h': h, 'st': [None] * NT} for h in range(1)]
            eq = [(early[h], kt) for h in range(1) for kt in range(8)]
            ei = 0
            for vc in range(2):
                vs = slice(vc * 512, vc * 512 + 512)
                wv = wpool.tile([P, NDM, 512], BF16, tag='w', name=f'wv{vc}')
                for o in range(NDM):
                    nc.sync.dma_start(wv[:, o, :], wvT[:, o, vs])
                for tt in range(NT):
                    psv = sppool.tile([P, 1024], F32, tag='sps',
                                      name=f'pv_{vc}_{tt}')[:, 0:512]
                    for o in range(NDM):
                        nc.tensor.matmul(psv[:],
                                         xsb[:, tt // 4, o,
                                             (tt % 4) * P:(tt % 4) * P + P],
                                         wv[:, o, :],
                                         start=(o == 0), stop=(o == NDM - 1))
                    vsb = evpool.tile([P, 512], BF16, tag='vsb',
                                      name=f'vsb_{vc}_{tt}')
                    nc.scalar.copy(vsb[:], psv[:])
                    nc.sync.dma_start(v_scr[tt][:, vs], vsb[:])
                    if (vc * NT + tt) % 2 == 1 and ei < len(eq):
                        ehd, ekt = eq[ei]
                        qk_tile(ehd, 0, ekt, stp=earlypool)
                        ei += 1

            # stage-1 pools closed: frees x (64KB/part), w, evict + 3 psum
            for p in reversed(_s1):
                p.__exit__(None, None, None)

            # ========== stage 2+3: attention fused with out-projection ======
            # (wo/osb pools open only after phase I frees the early pool)
            with tc.tile_pool(name='s2st', bufs=2) as stpool, \
                 tc.tile_pool(name='s2zt', bufs=1) as ztpool, \
                 tc.tile_pool(name='s2va', bufs=2) as vapool, \
                 tc.tile_pool(name='s2z', bufs=3) as zpool, \
                 tc.tile_pool(name='s2wo', bufs=2) as wopool, \
                 tc.tile_pool(name='s2os', bufs=2) as ospool:
                _stref.append(stpool)
                _ps2 = [tc.tile_pool(name='s2pz', bufs=3, space='PSUM'),
                        tc.tile_pool(name='s2pt', bufs=1, space='PSUM')]
                accpool, tppool = [p.__enter__() for p in _ps2]

                zT = [ztpool.tile([P, S], BF16, name=f'zT{h}') for h in range(HG)]

                def load_vau(h, kts):
                    vau = {}
                    for kt in kts:
                        va = vapool.tile([P, P + 1], BF16, tag=f'vau{kt}',
                                         name=f'vau_{h}_{kt}_{len(kts)}')
                        nc.sync.dma_start(va[:, 0:P],
                                          v_scr[kt][:, h * P:(h + 1) * P])
                        nc.vector.tensor_copy(va[:, P:P + 1], ones_sb[:])
                        vau[kt] = va
                    return vau

                pend_t = []

                def flush_t():
                    ph, pqa, pzsb = pend_t.pop(0)
                    ztp = tppool.tile([P, P], BF16, tag='ztp',
                                      name=f'ztp_{ph}_{pqa}')
                    nc.tensor.transpose(ztp[:], pzsb[:], ident[:])
                    nc.vector.tensor_copy(zT[ph][:, pqa * P:(pqa + 1) * P], ztp[:])

                def pv_chain(hd, qa):
                    h = hd['h']
                    qs = qa % 8
                    st, vau = hd['st'], hd['vau']
                    zps = accpool.tile([P, 512], F32, tag='acc',
                                       name=f'zps_{h}_{qa}')
                    for kt in range(qa + 1):
                        nc.tensor.matmul(zps[:, 0:P + 1],
                                         st[kt][:, qs * P:(qs + 1) * P],
                                         vau[kt][:],
                                         start=(kt == 0), stop=(kt == qa))
                    rcp = zpool.tile([P, 1], F32, tag='rcp', name=f'rcp_{h}_{qa}')
                    nc.vector.reciprocal(rcp[:], zps[:, P:P + 1])
                    zsb = zpool.tile([P, P], BF16, tag='zsb', name=f'zsb_{h}_{qa}')
                    nc.vector.tensor_scalar_mul(zsb[:], zps[:, 0:P], rcp[:])
                    # delay the transpose one step so the DVE epilogue
                    # latency hides under the next PV block's matmuls
                    pend_t.append((h, qa, zsb))
                    if len(pend_t) > 1:
                        flush_t()

                def proj_chain(ec, tt, wo, evict_act):
                    # out[tt, ec-slice] = sum_h zT_h[:, tt].T @ Wo_h[:, ec]
                    es = slice(ec * 512, ec * 512 + 512)
                    pso = accpool.tile([P, 512], F32, tag='acc',
                                       name=f'pso_{tt}_{ec}')
                    for h in range(HG):
                        nc.tensor.matmul(pso[:],
                                         zT[h][:, tt * P:(tt + 1) * P],
                                         wo[:, h, :],
                                         start=(h == 0), stop=(h == HG - 1))
                    osb = ospool.tile([P, 512], F32, tag='osb',
                                      name=f'osb_{tt}_{ec}')
                    if evict_act:
                        nc.scalar.copy(osb[:], pso[:])
                    else:
                        nc.vector.tensor_copy(osb[:], pso[:])
                    nc.sync.dma_start(out[tt][:, es], osb[:])

                def load_wo(ec):
                    wo = wopool.tile([P, HG, 512], BF16, tag='wo', name=f'wo{ec}')
                    es = slice(ec * 512, ec * 512 + 512)
                    for h in range(HG):
                        nc.sync.dma_start(wo[:, h, :], woT[:, h, es])
                    return wo

                # ---- phase I: first 1024 tokens, all heads ----
                for h in range(HG):
                    if h < 1:
                        # score tiles already computed during the V-part
                        hd = early[h]
                        hd['vau'] = load_vau(h, range(8))
                        for qa in range(8):
                            pv_chain(hd, qa)
                        continue
                    hd = {'h': h, 'vau': load_vau(h, range(8)), 'st': [None] * NT}
                    sched = [(0, 1, None), (2, 3, 0), (4, None, 1),
                             (5, None, 2), (6, None, 3), (7, None, 4)]
                    for a, b, qa in sched:
                        qk_tile(hd, 0, a)
                        if b is not None:
                            qk_tile(hd, 0, b)
                        if qa is not None:
                            pv_chain(hd, qa)
                    pv_chain(hd, 5)
                    pv_chain(hd, 6)
                    pv_chain(hd, 7)
                while pend_t:
                    flush_t()

                # ---- phase II: second 1024 tokens + projection of half 0 ----
                # proj chains (ec, tt<8) interleave, ec-major so wo double-
                # buffers; 4 proj chains per head
                projs = [(ec, tt) for ec in range(NC512) for tt in range(8)]
                wo_cur = load_wo(0)
                wo_tiles = {0: wo_cur}
                pi = 0

                def next_proj(evict_act=False):
                    nonlocal pi
                    ec, tt = projs[pi]
                    if ec + 1 < NC512 and tt == 4 and (ec + 1) not in wo_tiles:
                        wo_tiles[ec + 1] = load_wo(ec + 1)
                    proj_chain(ec, tt, wo_tiles[ec], evict_act)
                    pi += 1

                for h in range(HG):
                    hd = {'h': h, 'vau': load_vau(h, range(NT)), 'st': [None] * NT}
                    # prologue: 9 tiles before the first PV(qa=8) chain
                    qk_tile(hd, 1, 0)
                    qk_tile(hd, 1, 1)
                    next_proj()
                    qk_tile(hd, 1, 2)
                    qk_tile(hd, 1, 3)
                    next_proj()
                    qk_tile(hd, 1, 4)
                    qk_tile(hd, 1, 5)
                    next_proj()
                    qk_tile(hd, 1, 6)
                    qk_tile(hd, 1, 7)
                    qk_tile(hd, 1, 8)
                    next_proj()
                    for i in range(7):
                        qk_tile(hd, 1, 9 + i)
                        pv_chain(hd, 8 + i)
                    pv_chain(hd, 15)
                while pend_t:
                    flush_t()

                # ---- phase III: projection of half 1 (wo reloaded: the
                # 2-deep wo ring was recycled during phase II) ----
                for ec in range(NC512):
                    wo3 = load_wo(ec)
                    for tt in range(8, NT):
                        proj_chain(ec, tt, wo3, evict_act=(tt % 2 == 0))

                for p in reversed(_ps2):
                    p.__exit__(None, None, None)

            _early_cm.__exit__(None, None, None)
            _spp_cm.__exit__(None, None, None)

    nc.compile()
    return nc


def _host_inputs(x, Wqkv, Wo):
    """Build the 8 per-core input maps."""
    bf16 = ml_dtypes.bfloat16
    # RoPE tables (match reference: float32 math)
    inv_freq = (1.0 / (BASE ** (np.arange(0, DH, 2, dtype=np.float32) / DH))).astype(np.float32)
    t = np.arange(S, dtype=np.float32)
    freqs = np.einsum('i,j->ij', t, inv_freq).astype(np.float32)   # [S, 64]
    emb = np.concatenate([freqs, freqs], axis=-1)                   # [S, 128]
    cos = np.cos(emb).astype(np.float32)
    sin = np.sin(emb).astype(np.float32)
    cosT = np.ascontiguousarray(cos.T)                              # [128, S]
    sinT = np.ascontiguousarray(sin.T)
    sinP = sinT.copy()
    sinP[0:64] = -sinP[0:64]

    # triangular causal mask [128, 128] bf16: keep iff k_rel <= q_rel
    maskT = (np.arange(P)[:, None] <= np.arange(P)[None, :]).astype(bf16)
    onesb = np.ones((P, 1), dtype=bf16)
    identT = np.eye(P, dtype=np.float32).astype(bf16)

    in_maps = []
    for c in range(8):
        b, g = c // 2, c % 2
        heads = range(HG * g, HG * g + HG)
        x_b = x[b]                                       # [S, D]
        xT = np.ascontiguousarray(
            x_b.T.reshape(NDM, P, NC512, 512)
               .transpose(1, 2, 0, 3)).astype(bf16)      # [128, 4, 16, 512]
        # Q then K feature blocks, one per head in group:
        # [grp, 128, f4, o, 128] (f4-major so each fb is one contiguous DMA)
        blocks = [Wqkv[h * DH:(h + 1) * DH] for h in heads] + \
                 [Wqkv[D + h * DH:D + (h + 1) * DH] for h in heads]
        wqkT = np.stack([
            np.ascontiguousarray(
                blk.T.reshape(NDM, P, P).transpose(1, 0, 2))    # [128, 16, 128]
            for blk in blocks
        ])                                                       # [16, 128, 16, 128]
        wqkT = np.ascontiguousarray(
            wqkT.reshape(NFBG, 4, P, NDM, P).transpose(0, 2, 1, 3, 4)).astype(bf16)
        Wv = np.concatenate([Wqkv[2 * D + h * DH:2 * D + (h + 1) * DH] for h in heads])
        wvT = np.ascontiguousarray(
            Wv.T.reshape(NDM, P, HG * P).transpose(1, 0, 2)).astype(bf16)  # [128, 16, 1024]
        Wog = Wo[:, g * HG * DH:(g + 1) * HG * DH]               # [D, 1024]
        woT = np.ascontiguousarray(
            Wog.T.reshape(HG, P, D).transpose(1, 0, 2)).astype(bf16)       # [128, 8, D]
        in_maps.append({
            'xT': xT, 'wqkT': wqkT, 'wvT': wvT, 'woT': woT,
            'cosT': cosT, 'sinP': sinP, 'maskT': maskT, 'onesb': onesb,
            'identT': identT,
        })
    return in_maps


def kernel(x, Wqkv, Wo):
    from concourse.bass_utils import run_bass_kernel_spmd

    if 'nc' not in _CACHE:
        _CACHE['nc'] = _build_program()
    nc = _CACHE['nc']

    in_maps = _host_inputs(np.asarray(x, dtype=np.float32),
                           np.asarray(Wqkv, dtype=np.float32),
                           np.asarray(Wo, dtype=np.float32))
    res = run_bass_kernel_spmd(nc, in_maps, core_ids=list(range(8)))
    outs = [res.results[c]['out'].reshape(S, D) for c in range(8)]
    full = np.empty((B, S, D), dtype=np.float32)
    for b in range(B):
        full[b] = outs[2 * b] + outs[2 * b + 1]
    return full
